# revision 12
# baseline (speedup 1.0000x reference)
"""BiModalAttention Trainium2 kernel (v3 — interleaved pipeline).

Full-input contract: kernel(mode1, mode2) -> [S, B, 2D] float32.
mode1/mode2: [S=1024, B=32, D=1024] float32.

Reference computation per batch b (m1 = mode1[:, b, :], m2 = mode2[:, b, :]):
    C1 = m1 @ m2.T                  # [S, S]
    a1 = softmax_rows(C1) @ m2 * m1
    a2 = softmax_rows(C1.T) @ m1 * m2
    out[:, b, :] = concat([a1, a2], -1)

Sharding: batch dim across 8 NeuronCores (4 batch elements per core).

v3 changes vs v2 (569us baseline):
  - PE-dense interleaved emission. The PE queue never sits behind a
    softmax phase: round j emits
      A(j): C2-transpose/softmax t-loop of batch j  ||  AV-dir2 of batch j-1
      B(j): e2 i-loop of batch j                    ||  AV-dir1 of batch j
      C(j): scores matmuls of batch j+1 (PE-dense on their own)
    This removes both the ~110us of PE idle and most of the ~87us HAM
    half-clock penalty the phase-serial v2 paid (transposes don't count
    as PE activity for the HAM clock gate, so v2's softmax phases let the
    PE clock drop to 1.2GHz despite keeper matmuls).
  - Z1 accumulation moved from the e2 loop into A(j) (reads only c1+rm1),
    so invz1 is ready when B(j)'s dir-1 evacuations need it.
  - AV-dir2 evacuation split as ACT copy(scale=invz) + GpSimd gate-mult;
    dir-1 stays a single DVE scalar_tensor_tensor. Balances DVE in step A.
  - m1n/m2n stored bf16 in HBM (host-side cast; they only feed the bf16
    AV rhs/gate tiles) -> input DMA 64MB -> 48MB per core.
  - keeper matmuls dropped.
Scores stay fp32r: at N=512 moving they already run 1 cyc/row (bf16 rate)
and bf16 scores fail accuracy (softmax here is ~argmax; bf16 logit noise
flips near-tie selections: measured 6.9e-2 scale-rel error vs 2e-2 gate).
"""

import os
os.environ.setdefault("NEURON_RT_RESET_CORES", "1")
import time

import numpy as np

import concourse.bacc as bacc
import concourse.mybir as mybir
import concourse.tile as tile
from concourse.masks import make_identity
from concourse.bass_utils import run_bass_kernel_spmd

S = 1024
D = 1024
B = 32
N_CORES = 8
BPC = B // N_CORES          # batch elements per core
P = 128                     # partitions
NK = S // P                 # contraction tiles (8)
NI = S // P                 # s tiles (8)
CW = 512                    # AV d-chunk width
NCH = D // CW               # AV chunks (2)

f32 = mybir.dt.float32
f32r = mybir.dt.float32r
bf16 = mybir.dt.bfloat16
AX = mybir.AxisListType
ALU = mybir.AluOpType
ACTF = mybir.ActivationFunctionType


def _emit_t_loads(nc, sb, st, j, m1t, m2t):
    """Scores operands for batch j, d-major, f32->f32r, halved loads."""
    m1t_sb = st["m1t_sb"] = sb.tile([P, NK, S], f32r, tag="m1t", bufs=1,
                                    name=f"m1t_sb{j}")
    m2t_sb = st["m2t_sb"] = sb.tile([P, NK, S], f32r, tag="m2t", bufs=1,
                                    name=f"m2t_sb{j}")
    for lo in range(0, NK, 2):
        nc.gpsimd.dma_start(
            out=m1t_sb[:, lo:lo + 2, :],
            in_=m1t[j].rearrange("(k p) s -> p k s", p=P)[:, lo:lo + 2, :])
        nc.gpsimd.dma_start(
            out=m2t_sb[:, lo:lo + 2, :],
            in_=m2t[j].rearrange("(k p) s -> p k s", p=P)[:, lo:lo + 2, :])


def _emit_r_loads(nc, sb, st, j, m1n, m2n, c):
    """AV rhs/gate chunk c for batch j (bf16 HBM -> bf16 SBUF).

    Emission point matters: for j>=1 the pool slots being claimed are freed
    by AV-dir2(j-1) gate reads inside A(j), so this must be emitted after
    the t-iteration whose GpSimd gate-multiplies release them (t=3 for c0,
    t=7 for c1) or the scalar queue deadlocks behind the trigger's wait."""
    st.setdefault("r1", {})
    st.setdefault("r2", {})
    c0 = c * CW
    r2 = st["r2"][c] = sb.tile([P, NK, CW], bf16, tag="rhs", bufs=4,
                               name=f"r2_{j}_{c}")
    r1 = st["r1"][c] = sb.tile([P, NK, CW], bf16, tag="rhs", bufs=4,
                               name=f"r1_{j}_{c}")
    for lo in range(0, NK, 4):
        nc.scalar.dma_start(
            out=r2[:, lo:lo + 4, :],
            in_=m2n[j].rearrange("(k p) d -> p k d", p=P)[:, lo:lo + 4, c0:c0 + CW])
        nc.scalar.dma_start(
            out=r1[:, lo:lo + 4, :],
            in_=m1n[j].rearrange("(k p) d -> p k d", p=P)[:, lo:lo + 4, c0:c0 + CW])


def _emit_scores(nc, sb, ps, ident, st, j):
    """C1 = m1 @ m2.T for batch j: 16 fp32r PSUM groups -> c1 strips + rm1.
    Also computes z1/invz1 (ACT exp-accumulate over fresh c1 strips) and the
    rm1 partition-broadcast here, where PE work dominates DVE/ACT by 3x --
    keeping them out of the tightly-chained A step."""
    m1t_sb, m2t_sb = st["m1t_sb"], st["m2t_sb"]
    c1 = st["c1"] = []
    rm1 = st["rm1"] = sb.tile([P, NI], f32, tag="rm1", bufs=2, name=f"rm1_{j}")
    z1 = sb.tile([P, NI], f32, tag="z1", bufs=2, name=f"z1_{j}")
    rm1b = st["rm1b"] = sb.tile([P, S], f32, tag="rm1b", bufs=1,
                                name=f"rm1b_{j}")
    for i in range(NI):
        c1_i = sb.tile([P, S], f32, tag="c1", bufs=NI, name=f"c1_{j}_{i}")
        c1.append(c1_i)
        for n in range(2):
            pc = ps.tile([P, 512], f32, tag="c", bufs=4, name=f"pc{j}_{i}_{n}")
            for k in range(NK):
                nc.tensor.matmul(
                    pc,
                    m1t_sb[:, k, i * P:(i + 1) * P],
                    m2t_sb[:, k, n * 512:(n + 1) * 512],
                    start=(k == 0),
                    stop=(k == NK - 1),
                )
            nc.scalar.copy(out=c1_i[:, n * 512:(n + 1) * 512], in_=pc)
        nc.vector.tensor_reduce(rm1[:, i:i + 1], c1_i, axis=AX.X,
                                op=ALU.max, negate=True)
        scr = sb.tile([P, S], bf16, tag="scr", bufs=2, name=f"scr1_{j}_{i}")
        nc.scalar.activation(scr, c1_i, ACTF.Exp, bias=rm1[:, i:i + 1],
                             accum_out=z1[:, i:i + 1])
        _emit_bcast_col(nc, sb, ps, ident, rm1, i, rm1b, f"rm1b_{j}", "c")
    invz1 = st["invz1"] = sb.tile([P, NI], f32, tag="invz1", bufs=2,
                                  name=f"invz1_{j}")
    nc.vector.reciprocal(invz1, z1)


def _emit_bcast_col(nc, sb, ps, ident, rm_cols, i, rmb, nm, pstag):
    """One column of a partition-broadcast: RMB[t, i*P:(i+1)*P] = rm_cols[:, i]
    for all t. Emitted right after the reduce that produced column i, so the
    broadcast never forms a serial DVE backlog at a phase boundary."""
    xb = sb.tile([P, P], f32, tag="xb", bufs=3, name=f"{nm}_xb{i}")
    nc.vector.tensor_copy(xb, rm_cols[:, i:i + 1].broadcast_to([P, P]))
    ptb = ps.tile([P, 512], f32, tag=pstag, bufs=4, name=f"{nm}_ptb{i}")
    nc.tensor.transpose(ptb[:, 0:P], xb, ident)
    nc.scalar.copy(out=rmb[:, i * P:(i + 1) * P], in_=ptb[:, 0:P])


def _emit_av_group(nc, sb, ps, st, j, es, rhs, gate, invz, i, c, dbase, outp,
                   evac):
    """One AV output tile: psum = sum_k es[k][:, i-block].T @ rhs[:, k, :];
    out = (psum * invz[i]) * gate[:, i, :]."""
    pav = ps.tile([P, CW], f32, tag="av", bufs=4,
                  name=f"pav{j}_{c}_{i}_{dbase}")
    for k in range(NK):
        nc.tensor.matmul(
            pav,
            es[k][:, i * P:(i + 1) * P],
            rhs[:, k, :],
            start=(k == 0),
            stop=(k == NK - 1),
        )
    a_sb = sb.tile([P, CW], f32, tag="ao", bufs=6,
                   name=f"a{j}_{c}_{i}_{dbase}")
    if evac == "dve":
        nc.vector.scalar_tensor_tensor(
            a_sb, pav, invz[:, i:i + 1], gate[:, i, :],
            op0=ALU.mult, op1=ALU.mult)
    else:
        # ACT scale-copy + GpSimd gate multiply (keeps DVE free in step A)
        nc.scalar.activation(a_sb, pav, ACTF.Copy, scale=invz[:, i:i + 1])
        nc.gpsimd.tensor_mul(a_sb, a_sb, gate[:, i, :])
    nc.sync.dma_start(
        out=outp[j, i * P:(i + 1) * P, dbase + c * CW:dbase + (c + 1) * CW],
        in_=a_sb)


def _emit_A(nc, sb, ps, ident, st, prev, j, jprev, outp, r_load=None):
    """t-loop of batch j (C2 strips via PE transpose -> rm2, z2, e1) with
    AV-dir2 of batch j-1 interleaved into the PE stream. Also accumulates
    z1(j) (reads only c1+rm1)."""
    c1, rm1 = st["c1"], st["rm1"]
    rm1b = st["rm1b"]

    # AV-dir2 groups of the previous batch, c-major so r-chunk lifetimes
    # are half-pass: (c0,i0..7), (c1,i0..7); two groups per t-iteration.
    d2 = [(c, i) for c in range(NCH) for i in range(NI)] if prev else []

    e1 = st["e1"] = []
    rm2p = sb.tile([P, 2 * NK], f32, tag="rm2p", bufs=2, name=f"rm2p_{j}")
    rm2 = st["rm2"] = sb.tile([P, NK], f32, tag="rm2", bufs=2, name=f"rm2_{j}")
    z2p = sb.tile([P, 2 * NK], f32, tag="z2p", bufs=2, name=f"z2p_{j}")
    z2 = sb.tile([P, NK], f32, tag="z2", bufs=2, name=f"z2_{j}")
    rm2b = st["rm2b"] = sb.tile([P, S], f32, tag="rm2b", bufs=1,
                                name=f"rm2b_{j}")
    for t in range(NK):
        e1_t = sb.tile([P, S], bf16, tag="e1", bufs=NK, name=f"e1_{j}_{t}")
        e1.append(e1_t)
        epre = sb.tile([P, S], f32, tag="h", bufs=2, name=f"epre1_{j}_{t}")
        pts = []
        for g in range(2):
            pt = ps.tile([P, 512], f32, tag="c", bufs=4, name=f"pc2_{j}_{t}_{g}")
            pts.append(pt)
            for q in range(4):
                i = g * 4 + q
                nc.tensor.transpose(pt[:, q * P:(q + 1) * P],
                                    c1[i][:, t * P:(t + 1) * P], ident)
            nc.vector.tensor_reduce(rm2p[:, 2 * t + g:2 * t + g + 1], pt,
                                    axis=AX.X, op=ALU.max, negate=True)
        # AV-dir2(j-1): two groups keep the PE busy while DVE/ACT run softmax
        for (c, i) in d2[2 * t:2 * t + 2]:
            _emit_av_group(nc, sb, ps, prev, jprev, prev["e2"],
                           prev["r1"][c], prev["r2"][c], prev["invz2"],
                           i, c, D, outp, evac="act")
        nc.vector.tensor_tensor(rm2[:, t:t + 1], rm2p[:, 2 * t:2 * t + 1],
                                rm2p[:, 2 * t + 1:2 * t + 2], op=ALU.min)
        _emit_bcast_col(nc, sb, ps, ident, rm2, t, rm2b, f"rm2b_{j}", "av")
        for g in range(2):
            # Z2 partial straight from PSUM; fused shift on evacuation
            scrz = sb.tile([P, 512], bf16, tag="scr", bufs=2,
                           name=f"scrz_{j}_{t}_{g}")
            nc.scalar.activation(scrz, pts[g], ACTF.Exp, bias=rm2[:, t:t + 1],
                                 accum_out=z2p[:, 2 * t + g:2 * t + g + 1])
            nc.vector.tensor_add(epre[:, g * 512:(g + 1) * 512], pts[g],
                                 rm1b[:, g * 512:(g + 1) * 512])
        nc.vector.tensor_tensor(z2[:, t:t + 1], z2p[:, 2 * t:2 * t + 1],
                                z2p[:, 2 * t + 1:2 * t + 2], op=ALU.add)
        nc.scalar.activation(e1_t, epre, ACTF.Exp)
        # r(j) chunk loads, placed where their pool slots have just been
        # freed (see _emit_r_loads); for j=0 they were loaded in the prologue
        if r_load is not None and t in (3, NK - 1):
            r_load(0 if t == 3 else 1)

    invz2 = st["invz2"] = sb.tile([P, NI], f32, tag="invz2", bufs=2,
                                  name=f"invz2_{j}")
    nc.vector.reciprocal(invz2, z2)


def _emit_B(nc, sb, ps, ident, st, j, outp, last=False):
    """e2 i-loop of batch j with AV-dir1(j) interleaved (c-major passes)."""
    c1 = st["c1"]
    rm2b = st["rm2b"]
    e2 = st["e2"] = []
    for i in range(NI):
        e2_i = sb.tile([P, S], bf16, tag="e2", bufs=NI, name=f"e2_{j}_{i}")
        e2.append(e2_i)
        epre2 = sb.tile([P, S], f32, tag="h", bufs=2, name=f"epre2_{j}_{i}")
        nc.vector.tensor_add(epre2, c1[i], rm2b)
        nc.scalar.activation(e2_i, epre2, ACTF.Exp)
        # AV-dir1 chunk 0 rides along the i-loop
        _emit_av_group(nc, sb, ps, st, j, st["e1"], st["r2"][0], st["r1"][0],
                       st["invz1"], i, 0, 0, outp, evac="dve")
    if not last:
        for i in range(NI):
            _emit_av_group(nc, sb, ps, st, j, st["e1"], st["r2"][1],
                           st["r1"][1], st["invz1"], i, 1, 0, outp,
                           evac="dve")
    else:
        # final batch: its AV-dir2 has no later A step to ride in, so
        # interleave it with the dir-1 c1-pass (e2/invz2 are complete here)
        for i in range(NI):
            _emit_av_group(nc, sb, ps, st, j, st["e1"], st["r2"][1],
                           st["r1"][1], st["invz1"], i, 1, 0, outp,
                           evac="dve")
            for c in range(NCH):
                _emit_av_group(nc, sb, ps, st, j, st["e2"], st["r1"][c],
                               st["r2"][c], st["invz2"], i, c, D, outp,
                               evac="act")


def _build():
    nc = bacc.Bacc("TRN2", target_bir_lowering=False, debug=False,
                   num_devices=N_CORES)
    m1n = nc.dram_tensor("m1n", [BPC, S, D], bf16, kind="ExternalInput").ap()
    m2n = nc.dram_tensor("m2n", [BPC, S, D], bf16, kind="ExternalInput").ap()
    m1t = nc.dram_tensor("m1t", [BPC, D, S], f32, kind="ExternalInput").ap()
    m2t = nc.dram_tensor("m2t", [BPC, D, S], f32, kind="ExternalInput").ap()
    outp = nc.dram_tensor("out", [BPC, S, 2 * D], f32, kind="ExternalOutput").ap()

    with tile.TileContext(nc) as tc:
        with tc.tile_pool(name="consts", bufs=1) as consts, \
             tc.tile_pool(name="sb", bufs=1) as sb, \
             tc.tile_pool(name="ps", bufs=1, space="PSUM") as ps:
            ident = consts.tile([P, P], f32)
            make_identity(nc, ident)

            sts = [dict() for _ in range(BPC)]
            _emit_t_loads(nc, sb, sts[0], 0, m1t, m2t)
            _emit_r_loads(nc, sb, sts[0], 0, m1n, m2n, 0)
            _emit_r_loads(nc, sb, sts[0], 0, m1n, m2n, 1)
            _emit_scores(nc, sb, ps, ident, sts[0], 0)
            for j in range(BPC):
                if j + 1 < BPC:
                    _emit_t_loads(nc, sb, sts[j + 1], j + 1, m1t, m2t)
                prev = sts[j - 1] if j >= 1 else None
                r_load = ((lambda c, _j=j: _emit_r_loads(nc, sb, sts[_j], _j,
                                                         m1n, m2n, c))
                          if j >= 1 else None)
                _emit_A(nc, sb, ps, ident, sts[j], prev, j, j - 1, outp,
                        r_load=r_load)
                _emit_B(nc, sb, ps, ident, sts[j], j, outp,
                        last=(j == BPC - 1))
                if j + 1 < BPC:
                    _emit_scores(nc, sb, ps, ident, sts[j + 1], j + 1)
    nc.compile()
    return nc


_NC_CACHE = None


def _get_nc():
    global _NC_CACHE
    if _NC_CACHE is None:
        _NC_CACHE = _build()
    return _NC_CACHE


def kernel(mode1: np.ndarray, mode2: np.ndarray, _trace: bool = False,
           _result_box: dict | None = None) -> np.ndarray:
    import ml_dtypes
    mode1 = np.asarray(mode1, dtype=np.float32)
    mode2 = np.asarray(mode2, dtype=np.float32)

    m1n_all = np.ascontiguousarray(
        mode1.transpose(1, 0, 2)).astype(ml_dtypes.bfloat16)  # [B, S, D] bf16
    m2n_all = np.ascontiguousarray(
        mode2.transpose(1, 0, 2)).astype(ml_dtypes.bfloat16)
    m1t_all = np.ascontiguousarray(mode1.transpose(1, 2, 0))  # [B, D, S] f32
    m2t_all = np.ascontiguousarray(mode2.transpose(1, 2, 0))

    nc = _get_nc()
    in_maps = []
    for c in range(N_CORES):
        lo, hi = c * BPC, (c + 1) * BPC
        in_maps.append({
            "m1n": m1n_all[lo:hi],
            "m2n": m2n_all[lo:hi],
            "m1t": m1t_all[lo:hi],
            "m2t": m2t_all[lo:hi],
        })

    r = None
    last_err = None
    for attempt in range(3):
        try:
            r = run_bass_kernel_spmd(nc, in_maps, list(range(N_CORES)),
                                     trace=_trace)
            break
        except Exception as e:  # transient NRT exec-unit errors recover on retry
            last_err = e
            time.sleep(2.0)
    if r is None:
        raise last_err
    if _result_box is not None:
        _result_box["result"] = r

    out = np.empty((S, B, 2 * D), dtype=np.float32)
    for c in range(N_CORES):
        res = r.results[c]["out"]  # [BPC, S, 2D]
        out[:, c * BPC:(c + 1) * BPC, :] = res.transpose(1, 0, 2)
    return out


# revision 20
# speedup vs baseline: 1.0999x; 1.0999x over previous
"""BiModalAttention Trainium2 kernel (v6 — bf16 E-transposes, no fp32 C2).

Full-input contract: kernel(mode1, mode2) -> [S, B, 2D] float32.
mode1/mode2: [S=1024, B=32, D=1024] float32.

Reference per batch b (m1 = mode1[:, b, :], m2 = mode2[:, b, :]):
    C1 = m1 @ m2.T                  # [S, S]
    a1 = softmax_rows(C1) @ m2 * m1
    a2 = softmax_rows(C1.T) @ m1 * m2
    out[:, b, :] = concat([a1, a2], -1)

Sharding: batch dim across 8 NeuronCores (4 batch elements per core).

Pipeline (PE-dense interleaved emission; per-engine FIFO queues mean
emission order = execution order, so each round keeps real matmuls in
front of the PE while DVE/ACT run the softmax bookkeeping):
  C(j): scores(j+1) fp32r matmuls -> c1 strips; rm1 reduces; the rm1-shifted
        exp pass E1smaj = exp(c1 - rowmax) (bf16, KEPT) with z1 accum-out;
        W = exp(rowmax - M) prep + its partition-broadcast.
  A(j): E1T strips via *bf16* PE transposes of E1smaj (LDW at FWL rate, 1
        cyc/row — the old fp32 C2 transposes were LDW-bound at ~197ns) ||
        AV-dir2(j-1); rm2 = exact colmax recovered as ln(max_s E1T*W) + M;
        partition-broadcast of -rm2.
  B(j): e2u = exp(c1 - rm2) (free-dim shift, in-place add on c1) || AV-dir1
        chunk-0; z2 = colsum(e2u) via a ones-matmul in broadcast-row
        orientation; e2 = e2u * (1/z2) pre-normalized (so dir-2 evacuation
        is a plain gate multiply); AV-dir1 chunk-1 (+ dir-2 for the last
        batch).
Shift-consistency: softmax is invariant to any per-row shift applied
consistently to numerator and denominator; rm2 only needs to be within
~[-80, 0] of the true colmax (overflow/underflow), which the W-weighted
rowmax bound guarantees (exact colmax up to bf16 rounding of W, with a
1e-35 clamp guarding the e^-80 tail).
Scores stay fp32r: at N=512 they run 1 cyc/row (bf16 rate) and bf16
scores fail accuracy (softmax here is ~argmax; bf16 logit noise flips
near-tie selections: measured 6.9e-2 scale-rel vs the 2e-2 gate).
"""

import os
os.environ.setdefault("NEURON_RT_RESET_CORES", "1")
import time

import numpy as np

import concourse.bacc as bacc
import concourse.mybir as mybir
import concourse.tile as tile
from concourse.masks import make_identity
from concourse.bass_utils import run_bass_kernel_spmd
from concourse import bass_isa

S = 1024
D = 1024
B = 32
N_CORES = 8
BPC = B // N_CORES          # batch elements per core
P = 128                     # partitions
NK = S // P                 # contraction tiles (8)
NI = S // P                 # s tiles (8)
CW = 512                    # AV d-chunk width
NCH = D // CW               # AV chunks (2)

f32 = mybir.dt.float32
f32r = mybir.dt.float32r
bf16 = mybir.dt.bfloat16
AX = mybir.AxisListType
ALU = mybir.AluOpType
ACTF = mybir.ActivationFunctionType
RED = bass_isa.ReduceOp

_IDENT_BF = {}


def _emit_t_loads(nc, sb, st, j, m1t, m2t):
    """Scores operands for batch j, d-major, f32->f32r, quartered loads."""
    m1t_sb = st["m1t_sb"] = sb.tile([P, NK, S], f32r, tag="m1t", bufs=1,
                                    name=f"m1t_sb{j}")
    m2t_sb = st["m2t_sb"] = sb.tile([P, NK, S], f32r, tag="m2t", bufs=1,
                                    name=f"m2t_sb{j}")
    for lo in range(0, NK, 2):
        nc.gpsimd.dma_start(
            out=m1t_sb[:, lo:lo + 2, :],
            in_=m1t[j].rearrange("(k p) s -> p k s", p=P)[:, lo:lo + 2, :])
        nc.gpsimd.dma_start(
            out=m2t_sb[:, lo:lo + 2, :],
            in_=m2t[j].rearrange("(k p) s -> p k s", p=P)[:, lo:lo + 2, :])


def _emit_r_loads(nc, sb, st, j, m1n, m2n, c):
    """AV rhs/gate chunk c for batch j (bf16 HBM -> bf16 SBUF). For j>=1 the
    pool slots are freed by AV-dir2(j-1) reads inside A(j): emit after the
    t-iteration that releases them (t=3 for c0, t=7 for c1), else the scalar
    queue deadlocks behind the trigger's wait."""
    st.setdefault("r1", {})
    st.setdefault("r2", {})
    c0 = c * CW
    r2 = st["r2"][c] = sb.tile([P, NK, CW], bf16, tag="rhs", bufs=4,
                               name=f"r2_{j}_{c}")
    r1 = st["r1"][c] = sb.tile([P, NK, CW], bf16, tag="rhs", bufs=4,
                               name=f"r1_{j}_{c}")
    for lo in range(0, NK, 4):
        nc.scalar.dma_start(
            out=r2[:, lo:lo + 4, :],
            in_=m2n[j].rearrange("(k p) d -> p k d", p=P)[:, lo:lo + 4, c0:c0 + CW])
        nc.scalar.dma_start(
            out=r1[:, lo:lo + 4, :],
            in_=m1n[j].rearrange("(k p) d -> p k d", p=P)[:, lo:lo + 4, c0:c0 + CW])


def _bcast_to_rows(nc, sb, ps, identb, cols, rowsout, nm):
    """rowsout[p, i*P+u] = cols[u, i] for all p (bf16): free-dim broadcast
    copies + bf16 PE transposes (per-partition values -> broadcast rows).
    bf16 rounding of the broadcast shift values is harmless: the shift is
    applied consistently to numerator and denominator, so it cancels."""
    colsb = sb.tile([P, NI], bf16, tag="sm8b", bufs=2, name=f"{nm}_cb")
    nc.scalar.copy(out=colsb, in_=cols)
    pt = ps.tile([P, S], bf16, tag="tp", bufs=2, name=f"{nm}_pt")
    for i in range(NI):
        xb = sb.tile([P, P], bf16, tag="xb", bufs=3, name=f"{nm}_xb{i}")
        nc.vector.tensor_copy(xb, colsb[:, i:i + 1].broadcast_to([P, P]))
        nc.tensor.transpose(pt[:, i * P:(i + 1) * P], xb, identb)
    nc.scalar.copy(out=rowsout, in_=pt)


def _emit_scores(nc, sb, ps, ident, st, j):
    """C1 = m1 @ m2.T (fp32r); rm1; E1smaj = exp(c1 - rowmax) bf16 kept,
    with z1 accumulated on the same ACT pass; W = exp(rowmax - M) row-bcast."""
    m1t_sb, m2t_sb = st["m1t_sb"], st["m2t_sb"]
    c1 = st["c1"] = []
    e1s = st["e1s"] = []
    rm1 = st["rm1"] = sb.tile([P, NI], f32, tag="rm1", bufs=2, name=f"rm1_{j}")
    z1 = sb.tile([P, NI], f32, tag="z1", bufs=2, name=f"z1_{j}")
    for i in range(NI):
        c1_i = sb.tile([P, S], f32, tag="c1", bufs=NI, name=f"c1_{j}_{i}")
        c1.append(c1_i)
        for n in range(2):
            pc = ps.tile([P, 512], f32, tag="c", bufs=2, name=f"pc{j}_{i}_{n}")
            for k in range(NK):
                nc.tensor.matmul(
                    pc,
                    m1t_sb[:, k, i * P:(i + 1) * P],
                    m2t_sb[:, k, n * 512:(n + 1) * 512],
                    start=(k == 0),
                    stop=(k == NK - 1),
                )
            nc.scalar.copy(out=c1_i[:, n * 512:(n + 1) * 512], in_=pc)
        nc.vector.tensor_reduce(rm1[:, i:i + 1], c1_i, axis=AX.X,
                                op=ALU.max, negate=True)
        e1s_i = sb.tile([P, S], bf16, tag="e1s", bufs=NI, name=f"e1s_{j}_{i}")
        e1s.append(e1s_i)
        nc.scalar.activation(e1s_i, c1_i, ACTF.Exp, bias=rm1[:, i:i + 1],
                             accum_out=z1[:, i:i + 1])
    invz1 = st["invz1"] = sb.tile([P, NI], f32, tag="invz1", bufs=2,
                                  name=f"invz1_{j}")
    nc.vector.reciprocal(invz1, z1)
    # M = global max logit; W[s] = exp(rowmax[s] - M)  (rm1 = -rowmax)
    nmin = sb.tile([P, 1], f32, tag="sm", bufs=4, name=f"nmin_{j}")
    nc.vector.tensor_reduce(nmin, rm1, axis=AX.X, op=ALU.min, negate=True)
    ptm = ps.tile([P, 512], f32, tag="c", bufs=2, name=f"ptm_{j}")
    nc.tensor.transpose(ptm[0:1, 0:P], nmin, ident)
    nrow = sb.tile([1, P], f32, tag="sm", bufs=4, name=f"nrow_{j}")
    nc.scalar.copy(out=nrow, in_=ptm[0:1, 0:P])
    mrow = sb.tile([1, 1], f32, tag="sm", bufs=4, name=f"mrow_{j}")
    nc.vector.tensor_reduce(mrow, nrow, axis=AX.X, op=ALU.max)
    mrowb = sb.tile([1, P], f32, tag="sm", bufs=4, name=f"mrowb_{j}")
    nc.vector.tensor_copy(mrowb, mrow.broadcast_to([1, P]))
    ptm2 = ps.tile([P, 512], f32, tag="c", bufs=2, name=f"ptm2_{j}")
    nc.tensor.transpose(ptm2[0:P, 0:1], mrowb, ident[0:1, 0:1])
    mbc = st["mbc"] = sb.tile([P, 1], f32, tag="sm", bufs=4, name=f"mbc_{j}")
    nc.scalar.copy(out=mbc, in_=ptm2[0:P, 0:1])
    wcols = sb.tile([P, NI], f32, tag="sm8", bufs=4, name=f"wcols_{j}")
    nc.vector.tensor_tensor(wcols, rm1, mbc.broadcast_to([P, NI]), op=ALU.add)
    wexp = sb.tile([P, NI], f32, tag="sm8", bufs=4, name=f"wexp_{j}")
    nc.scalar.activation(wexp, wcols, ACTF.Exp, scale=-1.0)
    wball = st["wball"] = sb.tile([P, S], bf16, tag="wball", bufs=1,
                                  name=f"wball_{j}")
    _bcast_to_rows(nc, sb, ps, _IDENT_BF[id(nc)], wexp, wball, f"wb_{j}")


def _emit_av_group(nc, sb, ps, st, j, es, rhs, gate, invz, i, c, dbase, outp,
                   evac):
    """One AV output tile: psum = sum_k es[k][:, i-block].T @ rhs[:, k, :].
    dir-1 (invz not None): out = (psum * invz1[i]) * gate;
    dir-2 (invz None, es pre-normalized): out = psum * gate."""
    pav = ps.tile([P, CW], f32, tag="av", bufs=4,
                  name=f"pav{j}_{c}_{i}_{dbase}")
    for k in range(NK):
        nc.tensor.matmul(
            pav,
            es[k][:, i * P:(i + 1) * P],
            rhs[:, k, :],
            start=(k == 0),
            stop=(k == NK - 1),
        )
    a_sb = sb.tile([P, CW], f32, tag="ao", bufs=6,
                   name=f"a{j}_{c}_{i}_{dbase}")
    if invz is not None:
        nc.vector.scalar_tensor_tensor(
            a_sb, pav, invz[:, i:i + 1], gate[:, i, :],
            op0=ALU.mult, op1=ALU.mult)
    elif evac == "act":
        # ACT evacuation + GpSimd gate multiply keeps DVE free in step A
        nc.scalar.copy(out=a_sb, in_=pav)
        nc.gpsimd.tensor_mul(a_sb, a_sb, gate[:, i, :])
    else:
        nc.vector.tensor_tensor(a_sb, pav, gate[:, i, :], op=ALU.mult)
    nc.sync.dma_start(
        out=outp[j, i * P:(i + 1) * P, dbase + c * CW:dbase + (c + 1) * CW],
        in_=a_sb)


def _emit_A(nc, sb, ps, ident, st, prev, j, jprev, outp, r_load=None):
    """E1T strips via bf16 PE transposes of E1smaj, AV-dir2(j-1) interleaved;
    rm2 = exact colmax via ln(max_s E1T*W) + M; -rm2 row-broadcast."""
    e1s = st["e1s"]
    wball = st["wball"]
    identb = _IDENT_BF[id(nc)]
    d2 = [(c, i) for c in range(NCH) for i in range(NI)] if prev else []

    e1 = st["e1"] = []
    rmx = sb.tile([P, NK], f32, tag="rmx", bufs=2, name=f"rmx_{j}")
    for t in range(NK):
        e1_t = sb.tile([P, S], bf16, tag="e1", bufs=NK, name=f"e1_{j}_{t}")
        e1.append(e1_t)
        tp = ps.tile([P, S], bf16, tag="tp", bufs=2, name=f"tp_{j}_{t}")
        for i in range(NI):
            nc.tensor.transpose(tp[:, i * P:(i + 1) * P],
                                e1s[i][:, t * P:(t + 1) * P], identb)
        for (c, i) in d2[2 * t:2 * t + 2]:
            _emit_av_group(nc, sb, ps, prev, jprev, prev["e2"],
                           prev["r1"][c], prev["r2"][c], None,
                           i, c, D, outp, evac="act")
        nc.scalar.copy(out=e1_t, in_=tp)
        # rmx[t] = max_s E1T[t,s] * W[s]  (= exp(colmax - M))
        gsc = sb.tile([P, S], bf16, tag="gsc", bufs=1, name=f"gsc_{j}_{t}")
        nc.vector.tensor_tensor(gsc, e1_t, wball, op=ALU.mult)
        nc.vector.tensor_reduce(rmx[:, t:t + 1], gsc, axis=AX.X, op=ALU.max)
        if r_load is not None and t in (3, NK - 1):
            r_load(0 if t == 3 else 1)

    # nrm2[t] = -(ln(max(rmx, 1e-35)) + M); row-broadcast to [P, S]
    rmxc = sb.tile([P, NK], f32, tag="sm8", bufs=4, name=f"rmxc_{j}")
    nc.vector.tensor_scalar_max(rmxc, rmx, 1e-35)
    # ln(rmxc) upper bound from the f32 exponent field: ln(x) <= (e-126)*ln2
    # (integer-exact on HW; the ACT Ln table is not trusted on tiny inputs).
    # Slack <= ln2, absorbed by the ~80 margin of the exp shift.
    eint = sb.tile([P, NK], mybir.dt.uint32, tag="sm8", bufs=4,
                   name=f"eint_{j}")
    nc.vector.tensor_scalar(eint, rmxc.bitcast(mybir.dt.uint32), 23, None,
                            op0=ALU.logical_shift_right)
    ef = sb.tile([P, NK], f32, tag="sm8", bufs=4, name=f"ef_{j}")
    nc.vector.tensor_copy(ef, eint)
    mbcadj = sb.tile([P, 1], f32, tag="sm", bufs=4, name=f"mbcadj_{j}")
    nc.vector.tensor_scalar_add(mbcadj, st["mbc"], -87.33654475055556)
    nrm2 = sb.tile([P, NK], f32, tag="sm8", bufs=4, name=f"nrm2_{j}")
    nc.vector.scalar_tensor_tensor(
        nrm2, ef, -0.6931471805599453, mbcadj.broadcast_to([P, NK]),
        op0=ALU.mult, op1=ALU.subtract)
    nrm2b = st["nrm2b"] = sb.tile([P, S], bf16, tag="nrm2b", bufs=1,
                                  name=f"nrm2b_{j}")
    _bcast_to_rows(nc, sb, ps, _IDENT_BF[id(nc)], nrm2, nrm2b, f"n2_{j}")


def _emit_B(nc, sb, ps, ident, ones, st, j, outp, last=False):
    """e2u = exp(c1 - rm2) i-loop with AV-dir1 c0 interleaved; z2 via
    ones-matmul; e2 = e2u * invz2 (pre-normalized); AV-dir1 c1 pass
    (+ AV-dir2(j) when last)."""
    c1 = st["c1"]
    nrm2b = st["nrm2b"]
    e2u = []
    for i in range(NI):
        nc.vector.tensor_add(c1[i], c1[i], nrm2b)
        e2u_i = sb.tile([P, S], bf16, tag="e1s", bufs=NI, name=f"e2u_{j}_{i}")
        e2u.append(e2u_i)
        nc.scalar.activation(e2u_i, c1[i], ACTF.Exp)
        _emit_av_group(nc, sb, ps, st, j, st["e1"], st["r2"][0], st["r1"][0],
                       st["invz1"], i, 0, 0, outp, evac="dve")
    # z2(broadcast rows) = sum_s e2u; clamp; reciprocal; e2 = e2u * invz2
    z2b = sb.tile([P, S], f32, tag="z2b", bufs=1, name=f"z2b_{j}")
    for g in range(2):
        pz = ps.tile([P, 512], f32, tag="c", bufs=2, name=f"pz_{j}_{g}")
        for k in range(NK):
            nc.tensor.matmul(pz, ones, e2u[k][:, g * 512:(g + 1) * 512],
                             start=(k == 0), stop=(k == NK - 1))
        nc.scalar.copy(out=z2b[:, g * 512:(g + 1) * 512], in_=pz)
    nc.vector.tensor_scalar_max(z2b, z2b, 1e-30)
    nc.vector.reciprocal(z2b, z2b)
    e2 = st["e2"] = []
    for i in range(NI):
        e2_i = sb.tile([P, S], bf16, tag="e2", bufs=NI, name=f"e2_{j}_{i}")
        e2.append(e2_i)
        nc.vector.tensor_tensor(e2_i, e2u[i], z2b, op=ALU.mult)
    if not last:
        for i in range(NI):
            _emit_av_group(nc, sb, ps, st, j, st["e1"], st["r2"][1],
                           st["r1"][1], st["invz1"], i, 1, 0, outp,
                           evac="dve")
    else:
        for i in range(NI):
            _emit_av_group(nc, sb, ps, st, j, st["e1"], st["r2"][1],
                           st["r1"][1], st["invz1"], i, 1, 0, outp,
                           evac="dve")
            for c in range(NCH):
                _emit_av_group(nc, sb, ps, st, j, st["e2"], st["r1"][c],
                               st["r2"][c], None, i, c, D, outp, evac="dve")


def _build():
    nc = bacc.Bacc("TRN2", target_bir_lowering=False, debug=False,
                   num_devices=N_CORES)
    m1n = nc.dram_tensor("m1n", [BPC, S, D], bf16, kind="ExternalInput").ap()
    m2n = nc.dram_tensor("m2n", [BPC, S, D], bf16, kind="ExternalInput").ap()
    m1t = nc.dram_tensor("m1t", [BPC, D, S], f32, kind="ExternalInput").ap()
    m2t = nc.dram_tensor("m2t", [BPC, D, S], f32, kind="ExternalInput").ap()
    outp = nc.dram_tensor("out", [BPC, S, 2 * D], f32, kind="ExternalOutput").ap()

    with tile.TileContext(nc) as tc:
        with tc.tile_pool(name="consts", bufs=1) as consts, \
             tc.tile_pool(name="sb", bufs=1) as sb, \
             tc.tile_pool(name="ps", bufs=1, space="PSUM") as ps:
            ident = consts.tile([P, P], f32)
            make_identity(nc, ident)
            identb = consts.tile([P, P], bf16)
            nc.scalar.copy(out=identb, in_=ident)
            _IDENT_BF[id(nc)] = identb
            ones = consts.tile([P, P], bf16)
            nc.vector.memset(ones, 1.0)

            sts = [dict() for _ in range(BPC)]
            _emit_t_loads(nc, sb, sts[0], 0, m1t, m2t)
            _emit_r_loads(nc, sb, sts[0], 0, m1n, m2n, 0)
            _emit_r_loads(nc, sb, sts[0], 0, m1n, m2n, 1)
            _emit_scores(nc, sb, ps, ident, sts[0], 0)
            for j in range(BPC):
                if j + 1 < BPC:
                    _emit_t_loads(nc, sb, sts[j + 1], j + 1, m1t, m2t)
                prev = sts[j - 1] if j >= 1 else None
                r_load = ((lambda c, _j=j: _emit_r_loads(nc, sb, sts[_j], _j,
                                                         m1n, m2n, c))
                          if j >= 1 else None)
                _emit_A(nc, sb, ps, ident, sts[j], prev, j, j - 1, outp,
                        r_load=r_load)
                _emit_B(nc, sb, ps, ident, ones, sts[j], j, outp,
                        last=(j == BPC - 1))
                if j + 1 < BPC:
                    _emit_scores(nc, sb, ps, ident, sts[j + 1], j + 1)
    nc.compile()
    return nc


_NC_CACHE = None


def _get_nc():
    global _NC_CACHE
    if _NC_CACHE is None:
        _NC_CACHE = _build()
    return _NC_CACHE


def kernel(mode1: np.ndarray, mode2: np.ndarray, _trace: bool = False,
           _result_box: dict | None = None) -> np.ndarray:
    import ml_dtypes
    mode1 = np.asarray(mode1, dtype=np.float32)
    mode2 = np.asarray(mode2, dtype=np.float32)

    m1n_all = np.ascontiguousarray(
        mode1.transpose(1, 0, 2)).astype(ml_dtypes.bfloat16)  # [B, S, D] bf16
    m2n_all = np.ascontiguousarray(
        mode2.transpose(1, 0, 2)).astype(ml_dtypes.bfloat16)
    m1t_all = np.ascontiguousarray(mode1.transpose(1, 2, 0))  # [B, D, S] f32
    m2t_all = np.ascontiguousarray(mode2.transpose(1, 2, 0))

    nc = _get_nc()
    in_maps = []
    for c in range(N_CORES):
        lo, hi = c * BPC, (c + 1) * BPC
        in_maps.append({
            "m1n": m1n_all[lo:hi],
            "m2n": m2n_all[lo:hi],
            "m1t": m1t_all[lo:hi],
            "m2t": m2t_all[lo:hi],
        })

    r = None
    last_err = None
    for attempt in range(3):
        try:
            r = run_bass_kernel_spmd(nc, in_maps, list(range(N_CORES)),
                                     trace=_trace)
            break
        except Exception as e:  # transient NRT exec-unit errors recover on retry
            last_err = e
            time.sleep(2.0)
    if r is None:
        raise last_err
    if _result_box is not None:
        _result_box["result"] = r

    out = np.empty((S, B, 2 * D), dtype=np.float32)
    for c in range(N_CORES):
        res = r.results[c]["out"]  # [BPC, S, 2D]
        out[:, c * BPC:(c + 1) * BPC, :] = res.transpose(1, 0, 2)
    return out
